# revision 40
# baseline (speedup 1.0000x reference)
"""Bass/Trainium2 kernel for nn_DWAMiddleLayer (low-rank MoE weight-assembly).

Math:
    t[b,n,r]  = sum_a V[n,r,a] h_A[b,a]
    s[b,n,r]  = alpha[b,n] * t[b,n,r]
    h_t[b,c]  = sum_{n,r} s[b,n,r] U[n,c,r] + alpha@bE + h_A@W_base^T + b_base
    y = h_A + gamma*h_t ; out = LN(y)*ln_scale + ln_bias

Strategy: data-parallel over batch (BS=256/core), pool replicated, all matmul
operands fp8 (host-side cast+scale as in v1). v2 changes vs the 26.7us v1:
  * DMA: 5 transfers balanced over both HWDGE queues (SP + ACT), ordered so
    the o=0 pipeline inputs (hAT, VT_o0, alT, U_o0) land first.  v1 serialized
    950KB behind one queue and starved the PE until ~13us.
  * PE duty-cycle (HAM) warmup: the PE powers up at 4/8 duty (213ns per
    256-col matmul) and reaches 8/8 (107ns) only after ~3.4us of
    *uninterrupted* matmul activity.  Dummy matmuls run back-to-back from
    context entry until real data lands, so the real stream runs mostly warm.
  * PE order: all mm1 for o0/o1 first (DMA-gated), mm2 interleaved behind the
    alpha-multiplies, extras (W_base, eye-residual, bias) mid-stream when
    their (later) transfers land, bch0's last accumulation closes before
    bch1's so the LN epilogue overlaps the final matmuls.
  * alpha-multiply (the serial DVE chain, 8 x 512cols x 1.04ns): two chunks
    offloaded to GpSimd (Pool) so the DVE chain shortens to ~4us.
  * Epilogue split: bn_stats/recip/apply(bch0) on DVE, sqrt + apply(bch1) on
    ACT (Identity with per-partition scale=rstd, bias=-mu*rstd), outputs on
    both queues in parallel.
LN is scale-invariant so ht is normalized directly (eps' = eps/g_eff^2).
"""

import numpy as np

B, N, D_A, D_B, R = 2048, 512, 256, 256, 4
NC_COUNT = 8
BS = B // NC_COUNT  # 256
P = 128
LN_EPS = 1e-5

N_DUMMY = 14        # PE warmup matmuls (213ns each cold) before data lands

# dvt fp32-word layout (word offsets within the qs5[:,1536:] bitcast view)
EPS_OFF = 0    # eps/gamma_eff^2 fp32 [1]
EYE_OFF = 1    # eye128 bf16 [128] = 64 words
HAS_OFF = 65   # hAs bf16 [2,256] = 256 words
EP_OFF = 321   # ep bf16 [2,256] = 256 words (generic only)
DV_W_TRIV = 321
DV_W_GEN = 577

_cache = {}


def _build_nc(trivial_ep: bool):
    import concourse.mybir as mybir
    import concourse.tile as tile
    from concourse import bacc

    fp32 = mybir.dt.float32
    bf16 = mybir.dt.bfloat16
    f8 = mybir.dt.float8e4
    DR = mybir.MatmulPerfMode.DoubleRow
    DRSW = mybir.MatmulPerfMode.DoubleRowSwInterleave

    nc = bacc.Bacc("TRN2", target_bir_lowering=False)

    dv_b = 4 * (DV_W_TRIV if trivial_ep else DV_W_GEN)
    # inputs (f8 payloads packed per-partition; see make_in_maps).  All input
    # transfers ride ONE HWDGE queue (the 16 DMA engines are a shared pool,
    # so a second concurrent queue just splits bandwidth) as 6 back-to-back
    # transfers in consumption-priority order, each with its own completion
    # semaphore so the pipeline starts as soon as its chunk lands.
    d_qs0 = nc.dram_tensor("qs0", [P, 1536], f8, kind="ExternalInput")  # hAT|VT0
    d_qs1 = nc.dram_tensor("qs1", [P, 1024], f8, kind="ExternalInput")  # alT
    d_qs2 = nc.dram_tensor("qs2", [P, 2048], f8, kind="ExternalInput")  # U0|VT1
    d_qs3 = nc.dram_tensor("qs3", [P, 2048], f8, kind="ExternalInput")  # VT2|VT3
    d_qs4 = nc.dram_tensor("qs4", [P, 2048], f8, kind="ExternalInput")  # U1|U2
    d_qs5 = nc.dram_tensor("qs5", [P, 1536 + dv_b], f8, kind="ExternalInput")  # bE|Wb|dvt
    d_qs6 = nc.dram_tensor("qs6", [P, 1024], f8, kind="ExternalInput")  # U3
    d_out = nc.dram_tensor("out", [BS, D_A], fp32, kind="ExternalOutput")

    with tile.TileContext(nc) as tc:
        with (
            tc.tile_pool(name="persist", bufs=1) as persist,
            tc.tile_pool(name="spool", bufs=4) as spool,
            tc.tile_pool(name="sm", bufs=2) as sm,
            tc.tile_pool(name="pt", bufs=4, space="PSUM") as pt,
            tc.tile_pool(name="pacc", bufs=1, space="PSUM") as pacc,
            tc.tile_pool(name="pw", bufs=1, space="PSUM") as pw,
        ):
            # ---- SP queue: 7 priority-ordered input transfers ----
            qs0 = persist.tile([P, 1536], f8)
            nc.sync.dma_start(qs0, d_qs0[:])
            qs1 = persist.tile([P, 1024], f8)
            nc.sync.dma_start(qs1, d_qs1[:])
            qs2 = persist.tile([P, 2048], f8)
            nc.sync.dma_start(qs2, d_qs2[:])
            qs3 = persist.tile([P, 2048], f8)
            nc.sync.dma_start(qs3, d_qs3[:])
            qs4 = persist.tile([P, 2048], f8)
            nc.sync.dma_start(qs4, d_qs4[:])
            qs5 = persist.tile([P, 1536 + dv_b], f8)
            nc.sync.dma_start(qs5, d_qs5[:])
            qs6 = persist.tile([P, 1024], f8)
            nc.sync.dma_start(qs6, d_qs6[:])
            dvt = qs5[:, 1536:].bitcast(fp32)  # [P, DV_W]

            # ---- PE warmup source + small consts (GpSimd memsets) ----
            wz = persist.tile([P, 256], bf16)
            nc.gpsimd.memset(wz, 0.0)
            eps_col = persist.tile([P, 1], fp32)
            nc.gpsimd.memset(eps_col, LN_EPS)

            # ACT table preload happens before this first activation; it runs
            # during the DMA window so the epilogue Sqrt hits a warm table.
            warm = sm.tile([P, 1], fp32, tag="warm")
            nc.scalar.activation(
                warm, eps_col, mybir.ActivationFunctionType.Sqrt, bias=eps_col
            )

            # ---- views ----
            hAT = qs0[:, 0:512].rearrange("p (i b) -> p i b", i=2)  # [P,2,256]
            alT = qs1[:, 0:1024].rearrange("p (o b) -> p o b", o=4)  # [P,4,256]

            def vt_blk(o, r):  # mm1 lhsT block [P, 256] (hybrid DRSW layout)
                base = [qs0, qs2, qs3, qs3][o]
                off = [512, 1024, 0, 1024][o] + r * 256
                return base[:, off : off + 256].rearrange("p (j i) -> p j i", i=2)

            def u_blk(o, rp):  # mm2 rhs [P, 2, 256]
                base = [qs2, qs4, qs4, qs6][o]
                off = [0, 0, 1024, 0][o] + rp * 512
                return base[:, off : off + 512].rearrange("p (i c) -> p i c", i=2)

            bE = qs5[:, 0:1024].rearrange("p (op i c) -> p op i c", op=2, i=2)
            Wb = qs5[:, 1024:1536].rearrange("p (i c) -> p i c", i=2)
            epsp = dvt[:, EPS_OFF : EPS_OFF + 1]
            eye_b = dvt[:, EYE_OFF : EYE_OFF + 64].bitcast(bf16)  # [P,128]
            hAs = dvt[:, HAS_OFF : HAS_OFF + 256].bitcast(bf16).rearrange(
                "p (k c) -> p k c", k=2
            )
            if not trivial_ep:
                ep = dvt[:, EP_OFF : EP_OFF + 256].bitcast(bf16).rearrange(
                    "p (k c) -> p k c", k=2
                )

            # ---- PE HAM warmup: back-to-back dummy matmuls during DMA wait ----
            pwt = pw.tile([P, 256], fp32)
            for _ in range(N_DUMMY):
                nc.tensor.matmul(
                    pwt,
                    lhsT=wz[:, 0:128],
                    rhs=wz[:, 0:256],
                    start=True,
                    stop=True,
                    skip_group_check=True,
                )

            # ---- ht accumulators: one PSUM tile per batch half.  A single
            # shared tile would serialize the (cross-engine) epilogue readers
            # through the framework's last-accessor dependency tracking. ----
            ht0 = pacc.tile([P, D_B], fp32)
            ht1 = pacc.tile([P, D_B], fp32)
            hts = (ht0, ht1)
            started = [False, False]

            def acc(bch, lhsT, rhs, pmode, last=False):
                nc.tensor.matmul(
                    hts[bch],
                    lhsT=lhsT,
                    rhs=rhs,
                    start=(not started[bch]),
                    stop=last,
                    perf_mode=pmode,
                    skip_group_check=True,
                )
                started[bch] = True

            # ---- main pipeline ----
            # chunk index k = o*2+rp; t_ps/s8 tiles per chunk
            t_ps = {}
            s8 = {}

            def mm1(o, rp):
                tp = pt.tile([P, 2, BS], fp32, tag="t")
                t_ps[(o, rp)] = tp
                for rr in range(2):
                    nc.tensor.matmul(
                        tp[:, rr],
                        lhsT=vt_blk(o, rp * 2 + rr),
                        rhs=hAT,
                        start=True,
                        stop=True,
                        perf_mode=DRSW,
                    )

            def tt(o, rp, half=None):  # alpha-multiply (the serial DVE chain)
                if (o, rp) not in s8:
                    s = spool.tile([P, 2, BS], f8, tag="s", name=f"s8_{o}_{rp}")
                    s8[(o, rp)] = s
                s = s8[(o, rp)]
                sl = slice(None) if half is None else slice(half * P, (half + 1) * P)
                nc.vector.tensor_mul(
                    s[:, :, sl],
                    t_ps[(o, rp)][:, :, sl],
                    alT[:, o : o + 1, sl].to_broadcast(
                        (P, 2, BS if half is None else P)
                    ),
                )

            def mm2(o, rp, bchs=(0, 1), last=False):
                for bch in bchs:
                    lhsT = s8[(o, rp)][:, :, bch * P : (bch + 1) * P]
                    acc(bch, lhsT, u_blk(o, rp), DR, last=last)

            # o0 mm1 first (gated by qs1), rest pipelined behind the DMA
            # stream; TTs emitted right after their mm1 pair (DVE in-order).
            mm1(0, 0)
            mm1(0, 1)
            tt(0, 0)
            tt(0, 1)
            mm1(1, 0)  # qs2
            mm1(1, 1)
            tt(1, 0)
            tt(1, 1)
            mm2(0, 0)  # U_o0 in qs2
            mm2(0, 1)
            mm1(2, 0)  # qs3
            mm1(2, 1)
            tt(2, 0)
            tt(2, 1)
            mm1(3, 0)
            mm1(3, 1)
            tt(3, 0)
            # final-chunk TT split by batch half: bch0's last dependency
            # completes one DVE op earlier, overlapping its LN with bch1
            tt(3, 1, half=0)
            tt(3, 1, half=1)
            mm2(1, 0)  # U_o1 in qs4
            mm2(1, 1)
            # extras once qs5 lands: base(Wb), eye-residual, bias
            for bch in range(2):
                b_lhsT = hAT[:, :, bch * P : (bch + 1) * P]
                acc(bch, b_lhsT, Wb, DR)
                nc.tensor.matmul(
                    hts[bch],
                    lhsT=eye_b,
                    rhs=hAs[:, bch],
                    start=False,
                    stop=False,
                    skip_group_check=True,
                )
            for op in range(2):
                for bch in range(2):
                    a_lhsT = alT[:, op * 2 : (op + 1) * 2, bch * P : (bch + 1) * P]
                    acc(bch, a_lhsT, bE[:, op], DR)
            mm2(2, 0)
            mm2(2, 1)
            mm2(3, 0)  # U_o3 in qs6
            # close bch0 before bch1 so its LN overlaps the last matmuls
            mm2(3, 1, bchs=(0,), last=True)
            mm2(3, 1, bchs=(1,), last=True)

            # ---- epilogue: LN is scale-invariant, normalize ht directly
            # (y = g*ht + resid with resid already inside ht via the eye-mm;
            #  (y-mu_y)*rsqrt(var_y+eps) == (ht-mu_ht)*rsqrt(var_ht+eps/g^2))
            # Per-half stats tiles + per-half rstd via ACT Abs_reciprocal_sqrt
            # (1/sqrt(|var+eps|), positive arg): one ACT op per half, both
            # applies on DVE, both output DMAs on the SP queue -- minimizes
            # cross-engine hops and last-accessor serialization.
            ARS = mybir.ActivationFunctionType.Abs_reciprocal_sqrt
            st0 = sm.tile([P, 6], fp32, tag="st0")
            st1 = sm.tile([P, 6], fp32, tag="st1")
            mv0 = sm.tile([P, 2], fp32, tag="mv0")
            mv1 = sm.tile([P, 2], fp32, tag="mv1")
            rstd0 = sm.tile([P, 1], fp32, tag="rstd0")
            rstd1 = sm.tile([P, 1], fp32, tag="rstd1")
            out0 = sm.tile([P, D_A], fp32, tag="out0")
            out1 = sm.tile([P, D_A], fp32, tag="out1")

            nc.vector.bn_stats(st0, ht0)
            nc.vector.bn_aggr(mv0, st0)
            nc.scalar.activation(rstd0, mv0[:, 1:2], ARS, bias=epsp)
            nc.vector.bn_stats(st1, ht1)
            nc.vector.bn_aggr(mv1, st1)
            nc.scalar.activation(rstd1, mv1[:, 1:2], ARS, bias=epsp)

            for bch, (out_sb, htb, mvb, rstdb) in enumerate(
                ((out0, ht0, mv0, rstd0), (out1, ht1, mv1, rstd1))
            ):
                nc.vector.tensor_scalar(
                    out_sb,
                    htb,
                    scalar1=mvb[:, 0:1],
                    scalar2=rstdb,
                    op0=mybir.AluOpType.subtract,
                    op1=mybir.AluOpType.mult,
                )
                if not trivial_ep:
                    nc.vector.tensor_mul(
                        out_sb,
                        out_sb,
                        ep[:, 0:1, :].rearrange("p u c -> p (u c)").to_broadcast((P, D_A)),
                    )
                    nc.vector.tensor_add(
                        out_sb,
                        out_sb,
                        ep[:, 1:2, :].rearrange("p u c -> p (u c)").to_broadcast((P, D_A)),
                    )
                nc.sync.dma_start(d_out[bch * P : (bch + 1) * P, :], out_sb)

    nc.compile()
    return nc


def _get_nc(trivial_ep):
    if trivial_ep not in _cache:
        _cache[trivial_ep] = _build_nc(trivial_ep)
    return _cache[trivial_ep]


def make_in_maps(trivial_ep, **inputs):
    import ml_dtypes

    f8 = ml_dtypes.float8_e4m3
    q8 = lambda x: np.clip(x, -240, 240).astype(f8)

    f32 = lambda k: np.asarray(inputs[k], np.float32)
    h_A = f32("h_A")
    pool = f32("pool_vectors")
    alpha = f32("alpha")
    W_base = f32("W_base")
    b_base = f32("b_base").reshape(D_B)
    gamma = float(np.asarray(inputs["gamma"]).reshape(()))
    ln_s = f32("ln_scale").reshape(D_A)
    ln_b = f32("ln_bias").reshape(D_A)

    U = pool[:, : D_B * R].reshape(N, D_B, R)
    V = pool[:, D_B * R : D_B * R + R * D_A].reshape(N, R, D_A)
    bE = pool[:, D_B * R + R * D_A : D_B * R + R * D_A + D_B]

    V8 = q8(V * 16.0)  # [n, r, a]
    U8 = q8(U * 16.0)  # [n, c, r]
    bE8 = q8(bE * 256.0)  # [n, c]
    Wb8 = q8(W_base * 256.0)  # [c, a]
    g_eff = gamma / 256.0

    # ---- shared (pool-side) packing ----
    # VT blocks [P, o, r, 256]  (hybrid layout: [p, i, m])
    VTb = np.empty((P, 4, 4, 256), f8)
    V8v = V8.reshape(4, P, R, 2, P)  # [o, n, r, i, p]
    for o in range(4):
        for r in range(R):
            blk = V8v[o, :, r]  # [n=128(m), i, p]
            VTb[:, o, r] = blk.transpose(2, 1, 0).reshape(P, 256)  # p, i, m
    # U mm2-rhs [p, o, rp, rr, c]   (U8.reshape dims = (o, n_p, c, rp, rr))
    Ub = np.ascontiguousarray(U8.reshape(4, P, D_B, 2, 2).transpose(1, 0, 3, 4, 2))
    bEb = np.ascontiguousarray(
        bE8.reshape(2, 2, P, D_B).transpose(2, 0, 1, 3)
    )  # [p, op, i, c]
    Wbb = np.ascontiguousarray(
        Wb8.reshape(D_B, 2, P).transpose(2, 1, 0)
    )  # [p, i, c]

    qs3 = np.empty((P, 2048), f8)
    qs3[:, :1024] = VTb[:, 2].reshape(P, 1024)
    qs3[:, 1024:] = VTb[:, 3].reshape(P, 1024)
    qs4 = np.empty((P, 2048), f8)
    qs4[:, :1024] = Ub[:, 1].reshape(P, 1024)
    qs4[:, 1024:] = Ub[:, 2].reshape(P, 1024)
    qs6 = np.ascontiguousarray(Ub[:, 3].reshape(P, 1024))

    eye_words = (
        np.eye(P, dtype=np.float32).astype(ml_dtypes.bfloat16).view(np.float32)
    )  # [P, 64]

    dv_w = DV_W_TRIV if trivial_ep else DV_W_GEN
    in_maps = []
    for ci in range(NC_COUNT):
        sl = slice(ci * BS, (ci + 1) * BS)
        hA_c = h_A[sl]  # [256, 256]
        al_c = alpha[sl]  # [256, 512]
        hA8 = q8(hA_c)  # [b, a]
        al8 = q8(al_c)

        qs0 = np.empty((P, 1536), f8)
        # hAT [p, i, b] = hA8[b, i*128+p]
        qs0[:, :512] = hA8.reshape(BS, 2, P).transpose(2, 1, 0).reshape(P, 512)
        qs0[:, 512:] = VTb[:, 0].reshape(P, 1024)
        # alT [p, o, b] = al8[b, o*128+p]
        qs1 = np.ascontiguousarray(
            al8.reshape(BS, 4, P).transpose(2, 1, 0).reshape(P, 1024)
        )

        qs2 = np.empty((P, 2048), f8)
        qs2[:, :1024] = Ub[:, 0].reshape(P, 1024)
        qs2[:, 1024:] = VTb[:, 1].reshape(P, 1024)

        qs5 = np.empty((P, 1536 + 4 * dv_w), f8)
        qs5[:, :1024] = bEb.reshape(P, 1024)
        qs5[:, 1024:1536] = Wbb.reshape(P, 512)

        dve = np.zeros((P, dv_w), np.float32)
        dve[:, EPS_OFF] = LN_EPS / (g_eff * g_eff)
        dve[:, EYE_OFF : EYE_OFF + 64] = eye_words
        # hAs [p, bch, c] = (h_A[b(p,bch)] + gamma*b_base) / g_eff, bf16
        hAs_rows = (hA_c + gamma * b_base[None, :]) / g_eff
        hAs = hAs_rows.reshape(2, P, D_A)  # [bch, m, c] row index = b%128
        dve[:, HAS_OFF : HAS_OFF + 256] = (
            hAs.transpose(1, 0, 2).reshape(P, 512).astype(ml_dtypes.bfloat16)
        ).view(np.float32)
        if not trivial_ep:
            epb = np.empty((2, D_A), np.float32)
            epb[0] = ln_s
            epb[1] = ln_b
            dve[:, EP_OFF : EP_OFF + 256] = np.broadcast_to(
                epb.reshape(1, 512), (P, 512)
            ).astype(ml_dtypes.bfloat16).view(np.float32)
        qs5[:, 1536:] = dve.view(f8)

        in_maps.append(
            {
                "qs0": qs0,
                "qs1": qs1,
                "qs2": qs2,
                "qs3": qs3,
                "qs4": qs4,
                "qs5": qs5,
                "qs6": qs6,
            }
        )
    return in_maps


def run_kernel(trace=False, **inputs):
    from concourse.bass_utils import run_bass_kernel_spmd

    ln_s = np.asarray(inputs["ln_scale"], np.float32)
    ln_b = np.asarray(inputs["ln_bias"], np.float32)
    trivial_ep = bool(np.all(ln_s == 1.0) and np.all(ln_b == 0.0))
    nc = _get_nc(trivial_ep)
    in_maps = make_in_maps(trivial_ep, **inputs)
    res = run_bass_kernel_spmd(nc, in_maps, core_ids=list(range(NC_COUNT)), trace=trace)
    outs = [r["out"] for r in res.results]
    out = np.concatenate(outs, axis=0)
    return np.ascontiguousarray(out).astype(np.float32), res


def kernel(**inputs) -> np.ndarray:
    out, _ = run_kernel(trace=False, **inputs)
    return out


# revision 43
# speedup vs baseline: 1.0171x; 1.0171x over previous
"""Bass/Trainium2 kernel for nn_DWAMiddleLayer (low-rank MoE weight-assembly).

Math:
    t[b,n,r]  = sum_a V[n,r,a] h_A[b,a]
    s[b,n,r]  = alpha[b,n] * t[b,n,r]
    h_t[b,c]  = sum_{n,r} s[b,n,r] U[n,c,r] + alpha@bE + h_A@W_base^T + b_base
    y = h_A + gamma*h_t ; out = LN(y)*ln_scale + ln_bias

Strategy: data-parallel over batch (BS=256/core), pool replicated, all matmul
operands fp8 (host-side cast+scale as in v1). v2 changes vs the 26.7us v1:
  * DMA: 5 transfers balanced over both HWDGE queues (SP + ACT), ordered so
    the o=0 pipeline inputs (hAT, VT_o0, alT, U_o0) land first.  v1 serialized
    950KB behind one queue and starved the PE until ~13us.
  * PE duty-cycle (HAM) warmup: the PE powers up at 4/8 duty (213ns per
    256-col matmul) and reaches 8/8 (107ns) only after ~3.4us of
    *uninterrupted* matmul activity.  Dummy matmuls run back-to-back from
    context entry until real data lands, so the real stream runs mostly warm.
  * PE order: all mm1 for o0/o1 first (DMA-gated), mm2 interleaved behind the
    alpha-multiplies, extras (W_base, eye-residual, bias) mid-stream when
    their (later) transfers land, bch0's last accumulation closes before
    bch1's so the LN epilogue overlaps the final matmuls.
  * alpha-multiply (the serial DVE chain, 8 x 512cols x 1.04ns): two chunks
    offloaded to GpSimd (Pool) so the DVE chain shortens to ~4us.
  * Epilogue split: bn_stats/recip/apply(bch0) on DVE, sqrt + apply(bch1) on
    ACT (Identity with per-partition scale=rstd, bias=-mu*rstd), outputs on
    both queues in parallel.
LN is scale-invariant so ht is normalized directly (eps' = eps/g_eff^2).
"""

import numpy as np

B, N, D_A, D_B, R = 2048, 512, 256, 256, 4
NC_COUNT = 8
BS = B // NC_COUNT  # 256
P = 128
LN_EPS = 1e-5

N_DUMMY = 12        # PE warmup matmuls (213ns each cold) before data lands

# dvt fp32-word layout (word offsets within the qs5[:,1536:] bitcast view)
EPS_OFF = 0    # eps/gamma_eff^2 fp32 [1]
EYE_OFF = 1    # eye128 bf16 [128] = 64 words
HAS_OFF = 65   # hAs bf16 [2,256] = 256 words
EP_OFF = 321   # ep bf16 [2,256] = 256 words (generic only)
DV_W_TRIV = 321
DV_W_GEN = 577

_cache = {}


def _build_nc(trivial_ep: bool):
    import concourse.mybir as mybir
    import concourse.tile as tile
    from concourse import bacc

    fp32 = mybir.dt.float32
    bf16 = mybir.dt.bfloat16
    f8 = mybir.dt.float8e4
    DR = mybir.MatmulPerfMode.DoubleRow
    DRSW = mybir.MatmulPerfMode.DoubleRowSwInterleave

    nc = bacc.Bacc("TRN2", target_bir_lowering=False)

    dv_b = 4 * (DV_W_TRIV if trivial_ep else DV_W_GEN)
    # inputs (f8 payloads packed per-partition; see make_in_maps).  All input
    # transfers ride ONE HWDGE queue (the 16 DMA engines are a shared pool,
    # so a second concurrent queue just splits bandwidth) as 6 back-to-back
    # transfers in consumption-priority order, each with its own completion
    # semaphore so the pipeline starts as soon as its chunk lands.
    d_qs0 = nc.dram_tensor("qs0", [P, 1536], f8, kind="ExternalInput")  # hAT|VT0
    d_qs1 = nc.dram_tensor("qs1", [P, 1024], f8, kind="ExternalInput")  # alT
    d_qs2 = nc.dram_tensor("qs2", [P, 2048], f8, kind="ExternalInput")  # U0|VT1
    d_qs3 = nc.dram_tensor("qs3", [P, 2048], f8, kind="ExternalInput")  # VT2|VT3
    d_qs4 = nc.dram_tensor("qs4", [P, 2048], f8, kind="ExternalInput")  # U1|U2
    d_qs5 = nc.dram_tensor("qs5", [P, 1536 + dv_b], f8, kind="ExternalInput")  # bE|Wb|dvt
    d_qs6 = nc.dram_tensor("qs6", [P, 1024], f8, kind="ExternalInput")  # U3
    d_out = nc.dram_tensor("out", [BS, D_A], fp32, kind="ExternalOutput")

    with tile.TileContext(nc) as tc:
        with (
            tc.tile_pool(name="persist", bufs=1) as persist,
            tc.tile_pool(name="spool", bufs=4) as spool,
            tc.tile_pool(name="sm", bufs=2) as sm,
            tc.tile_pool(name="pt", bufs=4, space="PSUM") as pt,
            tc.tile_pool(name="pacc", bufs=1, space="PSUM") as pacc,
            tc.tile_pool(name="pw", bufs=1, space="PSUM") as pw,
        ):
            # ---- SP queue: 7 priority-ordered input transfers ----
            qs0 = persist.tile([P, 1536], f8)
            nc.sync.dma_start(qs0, d_qs0[:])
            qs1 = persist.tile([P, 1024], f8)
            nc.sync.dma_start(qs1, d_qs1[:])
            qs2 = persist.tile([P, 2048], f8)
            nc.sync.dma_start(qs2, d_qs2[:])
            qs3 = persist.tile([P, 2048], f8)
            nc.sync.dma_start(qs3, d_qs3[:])
            qs4 = persist.tile([P, 2048], f8)
            nc.sync.dma_start(qs4, d_qs4[:])
            qs5 = persist.tile([P, 1536 + dv_b], f8)
            nc.sync.dma_start(qs5, d_qs5[:])
            qs6 = persist.tile([P, 1024], f8)
            nc.sync.dma_start(qs6, d_qs6[:])
            dvt = qs5[:, 1536:].bitcast(fp32)  # [P, DV_W]

            # ---- PE warmup source + small consts (GpSimd memsets) ----
            wz = persist.tile([P, 256], bf16)
            nc.gpsimd.memset(wz, 0.0)
            eps_col = persist.tile([P, 1], fp32)
            nc.gpsimd.memset(eps_col, LN_EPS)

            # ACT table preload happens before this first activation; it runs
            # during the DMA window so the epilogue Sqrt hits a warm table.
            warm = sm.tile([P, 1], fp32, tag="warm")
            nc.scalar.activation(
                warm, eps_col, mybir.ActivationFunctionType.Sqrt, bias=eps_col
            )

            # ---- views ----
            hAT = qs0[:, 0:512].rearrange("p (i b) -> p i b", i=2)  # [P,2,256]
            alT = qs1[:, 0:1024].rearrange("p (o b) -> p o b", o=4)  # [P,4,256]

            def vt_blk(o, r):  # mm1 lhsT block [P, 256] (hybrid DRSW layout)
                base = [qs0, qs2, qs3, qs3][o]
                off = [512, 1024, 0, 1024][o] + r * 256
                return base[:, off : off + 256].rearrange("p (j i) -> p j i", i=2)

            def u_blk(o, rp):  # mm2 rhs [P, 2, 256]
                base = [qs2, qs4, qs4, qs6][o]
                off = [0, 0, 1024, 0][o] + rp * 512
                return base[:, off : off + 512].rearrange("p (i c) -> p i c", i=2)

            bE = qs5[:, 0:1024].rearrange("p (op i c) -> p op i c", op=2, i=2)
            Wb = qs5[:, 1024:1536].rearrange("p (i c) -> p i c", i=2)
            epsp = dvt[:, EPS_OFF : EPS_OFF + 1]
            eye_b = dvt[:, EYE_OFF : EYE_OFF + 64].bitcast(bf16)  # [P,128]
            hAs = dvt[:, HAS_OFF : HAS_OFF + 256].bitcast(bf16).rearrange(
                "p (k c) -> p k c", k=2
            )
            if not trivial_ep:
                ep = dvt[:, EP_OFF : EP_OFF + 256].bitcast(bf16).rearrange(
                    "p (k c) -> p k c", k=2
                )

            # ---- PE HAM warmup: back-to-back dummy matmuls during DMA wait ----
            pwt = pw.tile([P, 256], fp32)
            for _ in range(N_DUMMY):
                nc.tensor.matmul(
                    pwt,
                    lhsT=wz[:, 0:128],
                    rhs=wz[:, 0:256],
                    start=True,
                    stop=True,
                    skip_group_check=True,
                )

            # ---- ht accumulators: one PSUM tile per batch half.  A single
            # shared tile would serialize the (cross-engine) epilogue readers
            # through the framework's last-accessor dependency tracking. ----
            ht0 = pacc.tile([P, D_B], fp32)
            ht1 = pacc.tile([P, D_B], fp32)
            hts = (ht0, ht1)
            started = [False, False]

            def acc(bch, lhsT, rhs, pmode, last=False):
                nc.tensor.matmul(
                    hts[bch],
                    lhsT=lhsT,
                    rhs=rhs,
                    start=(not started[bch]),
                    stop=last,
                    perf_mode=pmode,
                    skip_group_check=True,
                )
                started[bch] = True

            # ---- main pipeline ----
            # chunk index k = o*2+rp; t_ps/s8 tiles per chunk
            t_ps = {}
            s8 = {}

            def mm1(o, rp):
                tp = pt.tile([P, 2, BS], fp32, tag="t")
                t_ps[(o, rp)] = tp
                for rr in range(2):
                    nc.tensor.matmul(
                        tp[:, rr],
                        lhsT=vt_blk(o, rp * 2 + rr),
                        rhs=hAT,
                        start=True,
                        stop=True,
                        perf_mode=DRSW,
                    )

            def tt(o, rp, half=None, rr=None):  # alpha-multiply (serial DVE chain)
                if (o, rp) not in s8:
                    s = spool.tile([P, 2, BS], f8, tag="s", name=f"s8_{o}_{rp}")
                    s8[(o, rp)] = s
                s = s8[(o, rp)]
                sl = slice(None) if half is None else slice(half * P, (half + 1) * P)
                rs = slice(None) if rr is None else slice(rr, rr + 1)
                nr = 2 if rr is None else 1
                nb = BS if half is None else P
                nc.vector.tensor_mul(
                    s[:, rs, sl],
                    t_ps[(o, rp)][:, rs, sl],
                    alT[:, o : o + 1, sl].to_broadcast((P, nr, nb)),
                )

            def mm2(o, rp, bchs=(0, 1), last=False):
                for bch in bchs:
                    lhsT = s8[(o, rp)][:, :, bch * P : (bch + 1) * P]
                    acc(bch, lhsT, u_blk(o, rp), DR, last=last)

            # o0 mm1 first (gated by qs1), rest pipelined behind the DMA
            # stream; TTs emitted right after their mm1 pair (DVE in-order).
            mm1(0, 0)
            mm1(0, 1)
            # chain head split per-rr: TT starts after ONE cold mm1 matmul
            tt(0, 0, rr=0)
            tt(0, 0, rr=1)
            tt(0, 1)
            mm1(1, 0)  # qs2
            mm1(1, 1)
            tt(1, 0)
            tt(1, 1)
            mm2(0, 0)  # U_o0 in qs2
            mm2(0, 1)
            mm1(2, 0)  # qs3
            mm1(2, 1)
            tt(2, 0)
            tt(2, 1)
            mm1(3, 0)
            mm1(3, 1)
            tt(3, 0)
            # final-chunk TT split by batch half: bch0's last dependency
            # completes one DVE op earlier, overlapping its LN with bch1
            tt(3, 1, half=0)
            tt(3, 1, half=1)
            mm2(1, 0)  # U_o1 in qs4
            mm2(1, 1)
            # extras once qs5 lands: base(Wb), eye-residual, bias
            for bch in range(2):
                b_lhsT = hAT[:, :, bch * P : (bch + 1) * P]
                acc(bch, b_lhsT, Wb, DR)
                nc.tensor.matmul(
                    hts[bch],
                    lhsT=eye_b,
                    rhs=hAs[:, bch],
                    start=False,
                    stop=False,
                    skip_group_check=True,
                )
            for op in range(2):
                for bch in range(2):
                    a_lhsT = alT[:, op * 2 : (op + 1) * 2, bch * P : (bch + 1) * P]
                    acc(bch, a_lhsT, bE[:, op], DR)
            mm2(2, 0)
            mm2(2, 1)
            mm2(3, 0)  # U_o3 in qs6
            # close bch0 before bch1 so its LN overlaps the last matmuls
            mm2(3, 1, bchs=(0,), last=True)
            mm2(3, 1, bchs=(1,), last=True)

            # ---- epilogue: LN is scale-invariant, normalize ht directly
            # (y = g*ht + resid with resid already inside ht via the eye-mm;
            #  (y-mu_y)*rsqrt(var_y+eps) == (ht-mu_ht)*rsqrt(var_ht+eps/g^2))
            # Per-half stats tiles + per-half rstd via ACT Abs_reciprocal_sqrt
            # (1/sqrt(|var+eps|), positive arg): one ACT op per half, both
            # applies on DVE, both output DMAs on the SP queue -- minimizes
            # cross-engine hops and last-accessor serialization.
            ARS = mybir.ActivationFunctionType.Abs_reciprocal_sqrt
            st0 = sm.tile([P, 6], fp32, tag="st0")
            st1 = sm.tile([P, 6], fp32, tag="st1")
            mv0 = sm.tile([P, 2], fp32, tag="mv0")
            mv1 = sm.tile([P, 2], fp32, tag="mv1")
            rstd0 = sm.tile([P, 1], fp32, tag="rstd0")
            rstd1 = sm.tile([P, 1], fp32, tag="rstd1")
            out0 = sm.tile([P, D_A], fp32, tag="out0")
            out1 = sm.tile([P, D_A], fp32, tag="out1")

            nc.vector.bn_stats(st0, ht0)
            nc.vector.bn_aggr(mv0, st0)
            nc.scalar.activation(rstd0, mv0[:, 1:2], ARS, bias=epsp)
            nc.vector.bn_stats(st1, ht1)
            nc.vector.bn_aggr(mv1, st1)
            nc.scalar.activation(rstd1, mv1[:, 1:2], ARS, bias=epsp)

            for bch, (out_sb, htb, mvb, rstdb) in enumerate(
                ((out0, ht0, mv0, rstd0), (out1, ht1, mv1, rstd1))
            ):
                nc.vector.tensor_scalar(
                    out_sb,
                    htb,
                    scalar1=mvb[:, 0:1],
                    scalar2=rstdb,
                    op0=mybir.AluOpType.subtract,
                    op1=mybir.AluOpType.mult,
                )
                if not trivial_ep:
                    nc.vector.tensor_mul(
                        out_sb,
                        out_sb,
                        ep[:, 0:1, :].rearrange("p u c -> p (u c)").to_broadcast((P, D_A)),
                    )
                    nc.vector.tensor_add(
                        out_sb,
                        out_sb,
                        ep[:, 1:2, :].rearrange("p u c -> p (u c)").to_broadcast((P, D_A)),
                    )
                nc.sync.dma_start(d_out[bch * P : (bch + 1) * P, :], out_sb)

    nc.compile()
    return nc


def _get_nc(trivial_ep):
    if trivial_ep not in _cache:
        _cache[trivial_ep] = _build_nc(trivial_ep)
    return _cache[trivial_ep]


def make_in_maps(trivial_ep, **inputs):
    import ml_dtypes

    f8 = ml_dtypes.float8_e4m3
    q8 = lambda x: np.clip(x, -240, 240).astype(f8)

    f32 = lambda k: np.asarray(inputs[k], np.float32)
    h_A = f32("h_A")
    pool = f32("pool_vectors")
    alpha = f32("alpha")
    W_base = f32("W_base")
    b_base = f32("b_base").reshape(D_B)
    gamma = float(np.asarray(inputs["gamma"]).reshape(()))
    ln_s = f32("ln_scale").reshape(D_A)
    ln_b = f32("ln_bias").reshape(D_A)

    U = pool[:, : D_B * R].reshape(N, D_B, R)
    V = pool[:, D_B * R : D_B * R + R * D_A].reshape(N, R, D_A)
    bE = pool[:, D_B * R + R * D_A : D_B * R + R * D_A + D_B]

    V8 = q8(V * 16.0)  # [n, r, a]
    U8 = q8(U * 16.0)  # [n, c, r]
    bE8 = q8(bE * 256.0)  # [n, c]
    Wb8 = q8(W_base * 256.0)  # [c, a]
    g_eff = gamma / 256.0

    # ---- shared (pool-side) packing ----
    # VT blocks [P, o, r, 256]  (hybrid layout: [p, i, m])
    VTb = np.empty((P, 4, 4, 256), f8)
    V8v = V8.reshape(4, P, R, 2, P)  # [o, n, r, i, p]
    for o in range(4):
        for r in range(R):
            blk = V8v[o, :, r]  # [n=128(m), i, p]
            VTb[:, o, r] = blk.transpose(2, 1, 0).reshape(P, 256)  # p, i, m
    # U mm2-rhs [p, o, rp, rr, c]   (U8.reshape dims = (o, n_p, c, rp, rr))
    Ub = np.ascontiguousarray(U8.reshape(4, P, D_B, 2, 2).transpose(1, 0, 3, 4, 2))
    bEb = np.ascontiguousarray(
        bE8.reshape(2, 2, P, D_B).transpose(2, 0, 1, 3)
    )  # [p, op, i, c]
    Wbb = np.ascontiguousarray(
        Wb8.reshape(D_B, 2, P).transpose(2, 1, 0)
    )  # [p, i, c]

    qs3 = np.empty((P, 2048), f8)
    qs3[:, :1024] = VTb[:, 2].reshape(P, 1024)
    qs3[:, 1024:] = VTb[:, 3].reshape(P, 1024)
    qs4 = np.empty((P, 2048), f8)
    qs4[:, :1024] = Ub[:, 1].reshape(P, 1024)
    qs4[:, 1024:] = Ub[:, 2].reshape(P, 1024)
    qs6 = np.ascontiguousarray(Ub[:, 3].reshape(P, 1024))

    eye_words = (
        np.eye(P, dtype=np.float32).astype(ml_dtypes.bfloat16).view(np.float32)
    )  # [P, 64]

    dv_w = DV_W_TRIV if trivial_ep else DV_W_GEN
    in_maps = []
    for ci in range(NC_COUNT):
        sl = slice(ci * BS, (ci + 1) * BS)
        hA_c = h_A[sl]  # [256, 256]
        al_c = alpha[sl]  # [256, 512]
        hA8 = q8(hA_c)  # [b, a]
        al8 = q8(al_c)

        qs0 = np.empty((P, 1536), f8)
        # hAT [p, i, b] = hA8[b, i*128+p]
        qs0[:, :512] = hA8.reshape(BS, 2, P).transpose(2, 1, 0).reshape(P, 512)
        qs0[:, 512:] = VTb[:, 0].reshape(P, 1024)
        # alT [p, o, b] = al8[b, o*128+p]
        qs1 = np.ascontiguousarray(
            al8.reshape(BS, 4, P).transpose(2, 1, 0).reshape(P, 1024)
        )

        qs2 = np.empty((P, 2048), f8)
        qs2[:, :1024] = Ub[:, 0].reshape(P, 1024)
        qs2[:, 1024:] = VTb[:, 1].reshape(P, 1024)

        qs5 = np.empty((P, 1536 + 4 * dv_w), f8)
        qs5[:, :1024] = bEb.reshape(P, 1024)
        qs5[:, 1024:1536] = Wbb.reshape(P, 512)

        dve = np.zeros((P, dv_w), np.float32)
        dve[:, EPS_OFF] = LN_EPS / (g_eff * g_eff)
        dve[:, EYE_OFF : EYE_OFF + 64] = eye_words
        # hAs [p, bch, c] = (h_A[b(p,bch)] + gamma*b_base) / g_eff, bf16
        hAs_rows = (hA_c + gamma * b_base[None, :]) / g_eff
        hAs = hAs_rows.reshape(2, P, D_A)  # [bch, m, c] row index = b%128
        dve[:, HAS_OFF : HAS_OFF + 256] = (
            hAs.transpose(1, 0, 2).reshape(P, 512).astype(ml_dtypes.bfloat16)
        ).view(np.float32)
        if not trivial_ep:
            epb = np.empty((2, D_A), np.float32)
            epb[0] = ln_s
            epb[1] = ln_b
            dve[:, EP_OFF : EP_OFF + 256] = np.broadcast_to(
                epb.reshape(1, 512), (P, 512)
            ).astype(ml_dtypes.bfloat16).view(np.float32)
        qs5[:, 1536:] = dve.view(f8)

        in_maps.append(
            {
                "qs0": qs0,
                "qs1": qs1,
                "qs2": qs2,
                "qs3": qs3,
                "qs4": qs4,
                "qs5": qs5,
                "qs6": qs6,
            }
        )
    return in_maps


def run_kernel(trace=False, **inputs):
    from concourse.bass_utils import run_bass_kernel_spmd

    ln_s = np.asarray(inputs["ln_scale"], np.float32)
    ln_b = np.asarray(inputs["ln_bias"], np.float32)
    trivial_ep = bool(np.all(ln_s == 1.0) and np.all(ln_b == 0.0))
    nc = _get_nc(trivial_ep)
    in_maps = make_in_maps(trivial_ep, **inputs)
    res = run_bass_kernel_spmd(nc, in_maps, core_ids=list(range(NC_COUNT)), trace=trace)
    outs = [r["out"] for r in res.results]
    out = np.concatenate(outs, axis=0)
    return np.ascontiguousarray(out).astype(np.float32), res


def kernel(**inputs) -> np.ndarray:
    out, _ = run_kernel(trace=False, **inputs)
    return out


# revision 48
# speedup vs baseline: 1.0233x; 1.0061x over previous
"""Bass/Trainium2 kernel for nn_DWAMiddleLayer (low-rank MoE weight-assembly).

Math:
    t[b,n,r]  = sum_a V[n,r,a] h_A[b,a]
    s[b,n,r]  = alpha[b,n] * t[b,n,r]
    h_t[b,c]  = sum_{n,r} s[b,n,r] U[n,c,r] + alpha@bE + h_A@W_base^T + b_base
    y = h_A + gamma*h_t ; out = LN(y)*ln_scale + ln_bias

Strategy: data-parallel over batch (BS=256/core), pool replicated, all matmul
operands fp8 (host-side cast+scale as in v1). v2 changes vs the 26.7us v1:
  * DMA: 5 transfers balanced over both HWDGE queues (SP + ACT), ordered so
    the o=0 pipeline inputs (hAT, VT_o0, alT, U_o0) land first.  v1 serialized
    950KB behind one queue and starved the PE until ~13us.
  * PE duty-cycle (HAM) warmup: the PE powers up at 4/8 duty (213ns per
    256-col matmul) and reaches 8/8 (107ns) only after ~3.4us of
    *uninterrupted* matmul activity.  Dummy matmuls run back-to-back from
    context entry until real data lands, so the real stream runs mostly warm.
  * PE order: all mm1 for o0/o1 first (DMA-gated), mm2 interleaved behind the
    alpha-multiplies, extras (W_base, eye-residual, bias) mid-stream when
    their (later) transfers land, bch0's last accumulation closes before
    bch1's so the LN epilogue overlaps the final matmuls.
  * alpha-multiply (the serial DVE chain, 8 x 512cols x 1.04ns): two chunks
    offloaded to GpSimd (Pool) so the DVE chain shortens to ~4us.
  * Epilogue split: bn_stats/recip/apply(bch0) on DVE, sqrt + apply(bch1) on
    ACT (Identity with per-partition scale=rstd, bias=-mu*rstd), outputs on
    both queues in parallel.
LN is scale-invariant so ht is normalized directly (eps' = eps/g_eff^2).
"""

import numpy as np

B, N, D_A, D_B, R = 2048, 512, 256, 256, 4
NC_COUNT = 8
BS = B // NC_COUNT  # 256
P = 128
LN_EPS = 1e-5

N_DUMMY = 12        # PE warmup matmuls (213ns each cold) before data lands

# dvt fp32-word layout (word offsets within the qs5[:,1536:] bitcast view)
EPS_OFF = 0    # eps/gamma_eff^2 fp32 [1]
EYE_OFF = 1    # eye128 bf16 [128] = 64 words
HAS_OFF = 65   # hAs bf16 [2,256] = 256 words
EP_OFF = 321   # ep bf16 [2,256] = 256 words (generic only)
DV_W_TRIV = 321
DV_W_GEN = 577

_cache = {}


def _build_nc(trivial_ep: bool):
    import concourse.mybir as mybir
    import concourse.tile as tile
    from concourse import bacc

    fp32 = mybir.dt.float32
    bf16 = mybir.dt.bfloat16
    f8 = mybir.dt.float8e4
    DR = mybir.MatmulPerfMode.DoubleRow
    DRSW = mybir.MatmulPerfMode.DoubleRowSwInterleave

    nc = bacc.Bacc("TRN2", target_bir_lowering=False)

    dv_b = 4 * (DV_W_TRIV if trivial_ep else DV_W_GEN)
    # inputs (f8 payloads packed per-partition; see make_in_maps).  All input
    # transfers ride ONE HWDGE queue (the 16 DMA engines are a shared pool,
    # so a second concurrent queue just splits bandwidth) as 6 back-to-back
    # transfers in consumption-priority order, each with its own completion
    # semaphore so the pipeline starts as soon as its chunk lands.
    d_qs0 = nc.dram_tensor("qs0", [P, 2048], f8, kind="ExternalInput")  # hAT|alT|VT0r01
    d_qs1 = nc.dram_tensor("qs1", [P, 512], f8, kind="ExternalInput")  # VT0r23
    d_qs2 = nc.dram_tensor("qs2", [P, 2048], f8, kind="ExternalInput")  # U0|VT1
    d_qs3 = nc.dram_tensor("qs3", [P, 2048], f8, kind="ExternalInput")  # VT2|VT3
    d_qs4 = nc.dram_tensor("qs4", [P, 2048], f8, kind="ExternalInput")  # U1|U2
    d_qs5 = nc.dram_tensor("qs5", [P, 1536 + dv_b], f8, kind="ExternalInput")  # bE|Wb|dvt
    d_qs6 = nc.dram_tensor("qs6", [P, 1024], f8, kind="ExternalInput")  # U3
    d_out = nc.dram_tensor("out", [BS, D_A], fp32, kind="ExternalOutput")

    with tile.TileContext(nc) as tc:
        with (
            tc.tile_pool(name="persist", bufs=1) as persist,
            tc.tile_pool(name="spool", bufs=4) as spool,
            tc.tile_pool(name="sm", bufs=2) as sm,
            tc.tile_pool(name="pt", bufs=4, space="PSUM") as pt,
            tc.tile_pool(name="pacc", bufs=1, space="PSUM") as pacc,
            tc.tile_pool(name="pw", bufs=1, space="PSUM") as pw,
        ):
            # ---- SP queue: 7 priority-ordered input transfers ----
            qs0 = persist.tile([P, 2048], f8)
            nc.sync.dma_start(qs0, d_qs0[:])
            qs1 = persist.tile([P, 512], f8)
            nc.sync.dma_start(qs1, d_qs1[:])
            qs2 = persist.tile([P, 2048], f8)
            nc.sync.dma_start(qs2, d_qs2[:])
            qs3 = persist.tile([P, 2048], f8)
            nc.sync.dma_start(qs3, d_qs3[:])
            qs4 = persist.tile([P, 2048], f8)
            nc.sync.dma_start(qs4, d_qs4[:])
            qs5 = persist.tile([P, 1536 + dv_b], f8)
            nc.sync.dma_start(qs5, d_qs5[:])
            qs6 = persist.tile([P, 1024], f8)
            nc.sync.dma_start(qs6, d_qs6[:])
            dvt = qs5[:, 1536:].bitcast(fp32)  # [P, DV_W]

            # ---- PE warmup source + small consts (GpSimd memsets) ----
            wz = persist.tile([P, 256], bf16)
            nc.gpsimd.memset(wz, 0.0)
            eps_col = persist.tile([P, 1], fp32)
            nc.gpsimd.memset(eps_col, LN_EPS)

            # ACT table preload happens before this first activation; it runs
            # during the DMA window so the epilogue Sqrt hits a warm table.
            warm = sm.tile([P, 1], fp32, tag="warm")
            nc.scalar.activation(
                warm, eps_col, mybir.ActivationFunctionType.Sqrt, bias=eps_col
            )

            # ---- views ----
            hAT = qs0[:, 0:512].rearrange("p (i b) -> p i b", i=2)  # [P,2,256]
            alT = qs0[:, 512:1536].rearrange("p (o b) -> p o b", o=4)  # [P,4,256]

            def vt_blk(o, r):  # mm1 lhsT block [P, 256] (hybrid DRSW layout)
                if o == 0:
                    base, off = (qs0, 1536) if r < 2 else (qs1, -512)
                else:
                    base, off = [None, qs2, qs3, qs3][o], [0, 1024, 0, 1024][o]
                off = off + r * 256
                return base[:, off : off + 256].rearrange("p (j i) -> p j i", i=2)

            def u_blk(o, rp):  # mm2 rhs [P, 2, 256]
                base = [qs2, qs4, qs4, qs6][o]
                off = [0, 0, 1024, 0][o] + rp * 512
                return base[:, off : off + 512].rearrange("p (i c) -> p i c", i=2)

            bE = qs5[:, 0:1024].rearrange("p (op i c) -> p op i c", op=2, i=2)
            Wb = qs5[:, 1024:1536].rearrange("p (i c) -> p i c", i=2)
            epsp = dvt[:, EPS_OFF : EPS_OFF + 1]
            eye_b = dvt[:, EYE_OFF : EYE_OFF + 64].bitcast(bf16)  # [P,128]
            hAs = dvt[:, HAS_OFF : HAS_OFF + 256].bitcast(bf16).rearrange(
                "p (k c) -> p k c", k=2
            )
            if not trivial_ep:
                ep = dvt[:, EP_OFF : EP_OFF + 256].bitcast(bf16).rearrange(
                    "p (k c) -> p k c", k=2
                )

            # ---- PE HAM warmup: back-to-back dummy matmuls during DMA wait ----
            pwt = pw.tile([P, 256], fp32)
            for _ in range(N_DUMMY):
                nc.tensor.matmul(
                    pwt,
                    lhsT=wz[:, 0:128],
                    rhs=wz[:, 0:256],
                    start=True,
                    stop=True,
                    skip_group_check=True,
                )

            # ---- ht accumulators: one PSUM tile per batch half.  A single
            # shared tile would serialize the (cross-engine) epilogue readers
            # through the framework's last-accessor dependency tracking. ----
            ht0 = pacc.tile([P, D_B], fp32)
            ht1 = pacc.tile([P, D_B], fp32)
            hts = (ht0, ht1)
            started = [False, False]

            def acc(bch, lhsT, rhs, pmode, last=False):
                nc.tensor.matmul(
                    hts[bch],
                    lhsT=lhsT,
                    rhs=rhs,
                    start=(not started[bch]),
                    stop=last,
                    perf_mode=pmode,
                    skip_group_check=True,
                )
                started[bch] = True

            # ---- main pipeline ----
            # chunk index k = o*2+rp; t_ps/s8 tiles per chunk
            t_ps = {}
            s8 = {}

            def mm1(o, rp):
                tp = pt.tile([P, 2, BS], fp32, tag="t")
                t_ps[(o, rp)] = tp
                for rr in range(2):
                    nc.tensor.matmul(
                        tp[:, rr],
                        lhsT=vt_blk(o, rp * 2 + rr),
                        rhs=hAT,
                        start=True,
                        stop=True,
                        perf_mode=DRSW,
                    )

            def tt(o, rp, half=None, rr=None):  # alpha-multiply (serial DVE chain)
                if (o, rp) not in s8:
                    s = spool.tile([P, 2, BS], f8, tag="s", name=f"s8_{o}_{rp}")
                    s8[(o, rp)] = s
                s = s8[(o, rp)]
                sl = slice(None) if half is None else slice(half * P, (half + 1) * P)
                rs = slice(None) if rr is None else slice(rr, rr + 1)
                nr = 2 if rr is None else 1
                nb = BS if half is None else P
                nc.vector.tensor_mul(
                    s[:, rs, sl],
                    t_ps[(o, rp)][:, rs, sl],
                    alT[:, o : o + 1, sl].to_broadcast((P, nr, nb)),
                )

            def mm2(o, rp, bchs=(0, 1), last=False):
                for bch in bchs:
                    lhsT = s8[(o, rp)][:, :, bch * P : (bch + 1) * P]
                    acc(bch, lhsT, u_blk(o, rp), DR, last=last)

            # o0 mm1 first (gated by qs1), rest pipelined behind the DMA
            # stream; TTs emitted right after their mm1 pair (DVE in-order).
            mm1(0, 0)
            mm1(0, 1)
            # chain head split per-rr: TT starts after ONE cold mm1 matmul
            tt(0, 0, rr=0)
            tt(0, 0, rr=1)
            tt(0, 1)
            mm1(1, 0)  # qs2
            mm1(1, 1)
            tt(1, 0)
            tt(1, 1)
            mm2(0, 0)  # U_o0 in qs2
            mm2(0, 1)
            mm1(2, 0)  # qs3
            mm1(2, 1)
            tt(2, 0)
            tt(2, 1)
            mm1(3, 0)
            mm1(3, 1)
            tt(3, 0)
            # final-chunk TT split by batch half: bch0's last dependency
            # completes one DVE op earlier, overlapping its LN with bch1
            tt(3, 1, half=0)
            tt(3, 1, half=1)
            mm2(1, 0)  # U_o1 in qs4
            mm2(1, 1)
            # extras once qs5 lands: base(Wb), eye-residual, bias
            for bch in range(2):
                b_lhsT = hAT[:, :, bch * P : (bch + 1) * P]
                acc(bch, b_lhsT, Wb, DR)
                nc.tensor.matmul(
                    hts[bch],
                    lhsT=eye_b,
                    rhs=hAs[:, bch],
                    start=False,
                    stop=False,
                    skip_group_check=True,
                )
            for op in range(2):
                for bch in range(2):
                    a_lhsT = alT[:, op * 2 : (op + 1) * 2, bch * P : (bch + 1) * P]
                    acc(bch, a_lhsT, bE[:, op], DR)
            mm2(2, 0)
            mm2(2, 1)
            mm2(3, 0)  # U_o3 in qs6
            # close bch0 before bch1 so its LN overlaps the last matmuls
            mm2(3, 1, bchs=(0,), last=True)
            mm2(3, 1, bchs=(1,), last=True)

            # ---- epilogue: LN is scale-invariant, normalize ht directly
            # (y = g*ht + resid with resid already inside ht via the eye-mm;
            #  (y-mu_y)*rsqrt(var_y+eps) == (ht-mu_ht)*rsqrt(var_ht+eps/g^2))
            # Per-half stats tiles + per-half rstd via ACT Abs_reciprocal_sqrt
            # (1/sqrt(|var+eps|), positive arg): one ACT op per half, both
            # applies on DVE, both output DMAs on the SP queue -- minimizes
            # cross-engine hops and last-accessor serialization.
            ARS = mybir.ActivationFunctionType.Abs_reciprocal_sqrt
            st0 = sm.tile([P, 6], fp32, tag="st0")
            st1 = sm.tile([P, 6], fp32, tag="st1")
            mv0 = sm.tile([P, 2], fp32, tag="mv0")
            mv1 = sm.tile([P, 2], fp32, tag="mv1")
            rstd0 = sm.tile([P, 1], fp32, tag="rstd0")
            rstd1 = sm.tile([P, 1], fp32, tag="rstd1")
            out0 = sm.tile([P, D_A], fp32, tag="out0")
            out1 = sm.tile([P, D_A], fp32, tag="out1")

            nc.vector.bn_stats(st0, ht0)
            nc.vector.bn_aggr(mv0, st0)
            nc.scalar.activation(rstd0, mv0[:, 1:2], ARS, bias=epsp)
            nc.vector.bn_stats(st1, ht1)
            nc.vector.bn_aggr(mv1, st1)
            nc.scalar.activation(rstd1, mv1[:, 1:2], ARS, bias=epsp)

            for bch, (out_sb, htb, mvb, rstdb) in enumerate(
                ((out0, ht0, mv0, rstd0), (out1, ht1, mv1, rstd1))
            ):
                nc.vector.tensor_scalar(
                    out_sb,
                    htb,
                    scalar1=mvb[:, 0:1],
                    scalar2=rstdb,
                    op0=mybir.AluOpType.subtract,
                    op1=mybir.AluOpType.mult,
                )
                if not trivial_ep:
                    nc.vector.tensor_mul(
                        out_sb,
                        out_sb,
                        ep[:, 0:1, :].rearrange("p u c -> p (u c)").to_broadcast((P, D_A)),
                    )
                    nc.vector.tensor_add(
                        out_sb,
                        out_sb,
                        ep[:, 1:2, :].rearrange("p u c -> p (u c)").to_broadcast((P, D_A)),
                    )
                # out1 rides the (idle) ACT queue so the two output transfers
                # overlap instead of queueing behind each other on SP
                q = nc.sync if bch == 0 else nc.scalar
                q.dma_start(d_out[bch * P : (bch + 1) * P, :], out_sb)

    nc.compile()
    return nc


def _get_nc(trivial_ep):
    if trivial_ep not in _cache:
        _cache[trivial_ep] = _build_nc(trivial_ep)
    return _cache[trivial_ep]


def make_in_maps(trivial_ep, **inputs):
    import ml_dtypes

    f8 = ml_dtypes.float8_e4m3
    q8 = lambda x: np.clip(x, -240, 240).astype(f8)

    f32 = lambda k: np.asarray(inputs[k], np.float32)
    h_A = f32("h_A")
    pool = f32("pool_vectors")
    alpha = f32("alpha")
    W_base = f32("W_base")
    b_base = f32("b_base").reshape(D_B)
    gamma = float(np.asarray(inputs["gamma"]).reshape(()))
    ln_s = f32("ln_scale").reshape(D_A)
    ln_b = f32("ln_bias").reshape(D_A)

    U = pool[:, : D_B * R].reshape(N, D_B, R)
    V = pool[:, D_B * R : D_B * R + R * D_A].reshape(N, R, D_A)
    bE = pool[:, D_B * R + R * D_A : D_B * R + R * D_A + D_B]

    V8 = q8(V * 16.0)  # [n, r, a]
    U8 = q8(U * 16.0)  # [n, c, r]
    bE8 = q8(bE * 256.0)  # [n, c]
    Wb8 = q8(W_base * 256.0)  # [c, a]
    g_eff = gamma / 256.0

    # ---- shared (pool-side) packing ----
    # VT blocks [P, o, r, 256]  (hybrid layout: [p, i, m])
    VTb = np.empty((P, 4, 4, 256), f8)
    V8v = V8.reshape(4, P, R, 2, P)  # [o, n, r, i, p]
    for o in range(4):
        for r in range(R):
            blk = V8v[o, :, r]  # [n=128(m), i, p]
            VTb[:, o, r] = blk.transpose(2, 1, 0).reshape(P, 256)  # p, i, m
    # U mm2-rhs [p, o, rp, rr, c]   (U8.reshape dims = (o, n_p, c, rp, rr))
    Ub = np.ascontiguousarray(U8.reshape(4, P, D_B, 2, 2).transpose(1, 0, 3, 4, 2))
    bEb = np.ascontiguousarray(
        bE8.reshape(2, 2, P, D_B).transpose(2, 0, 1, 3)
    )  # [p, op, i, c]
    Wbb = np.ascontiguousarray(
        Wb8.reshape(D_B, 2, P).transpose(2, 1, 0)
    )  # [p, i, c]

    qs3 = np.empty((P, 2048), f8)
    qs3[:, :1024] = VTb[:, 2].reshape(P, 1024)
    qs3[:, 1024:] = VTb[:, 3].reshape(P, 1024)
    qs4 = np.empty((P, 2048), f8)
    qs4[:, :1024] = Ub[:, 1].reshape(P, 1024)
    qs4[:, 1024:] = Ub[:, 2].reshape(P, 1024)
    qs6 = np.ascontiguousarray(Ub[:, 3].reshape(P, 1024))

    eye_words = (
        np.eye(P, dtype=np.float32).astype(ml_dtypes.bfloat16).view(np.float32)
    )  # [P, 64]

    dv_w = DV_W_TRIV if trivial_ep else DV_W_GEN
    in_maps = []
    for ci in range(NC_COUNT):
        sl = slice(ci * BS, (ci + 1) * BS)
        hA_c = h_A[sl]  # [256, 256]
        al_c = alpha[sl]  # [256, 512]
        hA8 = q8(hA_c)  # [b, a]
        al8 = q8(al_c)

        qs0 = np.empty((P, 2048), f8)
        # hAT [p, i, b] = hA8[b, i*128+p]
        qs0[:, :512] = hA8.reshape(BS, 2, P).transpose(2, 1, 0).reshape(P, 512)
        # alT [p, o, b] = al8[b, o*128+p]
        qs0[:, 512:1536] = al8.reshape(BS, 4, P).transpose(2, 1, 0).reshape(P, 1024)
        qs0[:, 1536:] = VTb[:, 0, :2].reshape(P, 512)
        qs1 = np.ascontiguousarray(VTb[:, 0, 2:].reshape(P, 512))

        qs2 = np.empty((P, 2048), f8)
        qs2[:, :1024] = Ub[:, 0].reshape(P, 1024)
        qs2[:, 1024:] = VTb[:, 1].reshape(P, 1024)

        qs5 = np.empty((P, 1536 + 4 * dv_w), f8)
        qs5[:, :1024] = bEb.reshape(P, 1024)
        qs5[:, 1024:1536] = Wbb.reshape(P, 512)

        dve = np.zeros((P, dv_w), np.float32)
        dve[:, EPS_OFF] = LN_EPS / (g_eff * g_eff)
        dve[:, EYE_OFF : EYE_OFF + 64] = eye_words
        # hAs [p, bch, c] = (h_A[b(p,bch)] + gamma*b_base) / g_eff, bf16
        hAs_rows = (hA_c + gamma * b_base[None, :]) / g_eff
        hAs = hAs_rows.reshape(2, P, D_A)  # [bch, m, c] row index = b%128
        dve[:, HAS_OFF : HAS_OFF + 256] = (
            hAs.transpose(1, 0, 2).reshape(P, 512).astype(ml_dtypes.bfloat16)
        ).view(np.float32)
        if not trivial_ep:
            epb = np.empty((2, D_A), np.float32)
            epb[0] = ln_s
            epb[1] = ln_b
            dve[:, EP_OFF : EP_OFF + 256] = np.broadcast_to(
                epb.reshape(1, 512), (P, 512)
            ).astype(ml_dtypes.bfloat16).view(np.float32)
        qs5[:, 1536:] = dve.view(f8)

        in_maps.append(
            {
                "qs0": qs0,
                "qs1": qs1,
                "qs2": qs2,
                "qs3": qs3,
                "qs4": qs4,
                "qs5": qs5,
                "qs6": qs6,
            }
        )
    return in_maps


def run_kernel(trace=False, **inputs):
    from concourse.bass_utils import run_bass_kernel_spmd

    ln_s = np.asarray(inputs["ln_scale"], np.float32)
    ln_b = np.asarray(inputs["ln_bias"], np.float32)
    trivial_ep = bool(np.all(ln_s == 1.0) and np.all(ln_b == 0.0))
    nc = _get_nc(trivial_ep)
    in_maps = make_in_maps(trivial_ep, **inputs)
    res = run_bass_kernel_spmd(nc, in_maps, core_ids=list(range(NC_COUNT)), trace=trace)
    outs = [r["out"] for r in res.results]
    out = np.concatenate(outs, axis=0)
    return np.ascontiguousarray(out).astype(np.float32), res


def kernel(**inputs) -> np.ndarray:
    out, _ = run_kernel(trace=False, **inputs)
    return out
